# revision 1
# baseline (speedup 1.0000x reference)
"""AttentionLSTM Trainium2 kernel, 8-core SPMD.

Strategy: tensor-parallel over the 4H gate dimension. Core k owns H-slice
hd in [128k, 128(k+1)) of each of the four gates (512 act columns, laid out
[i_k | f_k | o_k | g_k]) and the matching slices of the c/h state.

Per timestep, one AllGather carries [partial_scores(16 f32) | hT-slice(128)]
per batch row (bf16): the gathered hT blocks form exactly the lhsT k-tiles
for h@Wh, and the 8 partial-score blocks sum to the full attention scores
on every core.  The attention's contribution to the gate pre-activations
is computed without materializing attn: the prologue builds
B[n,p,j] = sum_h Af[n,h,p] * Wattn[h,j]  (j-sharded), and each step does
act += sum_p w[n,p] * B[n,p,j] as 16 diagonal matmuls diag(w_p) @ B_p
accumulated into the same PSUM bank as x@Wx (prefetched during the
AllGather window) and h@Wh.

All activations go through the exp/tanh table only (sigmoid(z) =
0.5*tanh(z/2)+0.5 with the g-gate weight columns pre-doubled host-side), so
the scalar engine never swaps activation tables.  The payload hT is formed
as o^T (.) tanh(c)^T from two PE transposes, skipping the h_new dependency,
and ships together with the partial scores in a single DMA.
"""
import numpy as np

from concourse import bacc, tile
from concourse import mybir
from concourse.bass_utils import run_bass_kernel_spmd

N, T, D, H = 128, 64, 1024, 1024
P16 = 16          # attention positions (4x4)
NC = 8            # cores
HS = H // NC      # 128, per-core H slice
JS = 4 * HS       # 512, per-core act columns
KT = D // 128     # 8 k-tiles
PW = 2 * P16 + 128  # payload cols: 16 f32 partial scores (as 32 bf16) + 128 hT

F32 = mybir.dt.float32
BF16 = mybir.dt.bfloat16
BF16_NP = mybir.dt.np(mybir.dt.bfloat16)
RG = [list(range(NC))]

_nc_cache = None


def _build():
    nc = bacc.Bacc("TRN2", target_bir_lowering=False, debug=False, num_devices=NC)

    xT_d = nc.dram_tensor("xT", [T, 128, KT, N], BF16, kind="ExternalInput").ap()
    wx_d = nc.dram_tensor("wx", [128, KT, JS], BF16, kind="ExternalInput").ap()
    wh_d = nc.dram_tensor("wh", [128, KT, JS], BF16, kind="ExternalInput").ap()
    wattn_d = nc.dram_tensor("wattn", [128, KT, JS], BF16, kind="ExternalInput").ap()
    b_d = nc.dram_tensor("bvec", [1, JS], BF16, kind="ExternalInput").ap()
    ident_d = nc.dram_tensor("ident", [128, 128], BF16, kind="ExternalInput").ap()
    afn_d = nc.dram_tensor("afn", [128, P16, HS], BF16, kind="ExternalInput").ap()
    afT_d = nc.dram_tensor("afT", [128, KT, P16, N], BF16, kind="ExternalInput").ap()
    h0_d = nc.dram_tensor("h0", [128, HS], F32, kind="ExternalInput").ap()
    out_d = nc.dram_tensor("out", [T, HS, N], BF16, kind="ExternalOutput").ap()

    with tile.TileContext(nc) as tc:
        with (
            tc.tile_pool(name="const", bufs=1) as cp,
            tc.tile_pool(name="state", bufs=2) as sp,
            tc.tile_pool(name="work", bufs=2) as wp,
            tc.tile_pool(name="xpool", bufs=3) as xp,
            tc.tile_pool(name="paypool", bufs=2) as yp,
            tc.tile_pool(name="psum", bufs=2, space="PSUM") as pp,
            tc.tile_pool(name="tpsum", bufs=2, space="PSUM") as tp,
            tc.tile_pool(name="dram", bufs=2, space="DRAM") as dp,
        ):
            # ---------------- constants (direct bf16 loads) ----------------
            wx_b = cp.tile([128, KT, JS], BF16, name="wx_b")
            nc.sync.dma_start(out=wx_b[:], in_=wx_d[:])
            wh_b = cp.tile([128, KT, JS], BF16, name="wh_b")
            nc.sync.dma_start(out=wh_b[:], in_=wh_d[:])
            b_b = cp.tile([1, JS], BF16, name="b_b")
            nc.sync.dma_start(out=b_b[:], in_=b_d[:])
            ident_b = cp.tile([128, 128], BF16, name="ident_b")
            nc.sync.dma_start(out=ident_b[:], in_=ident_d[:])
            afn_b = cp.tile([128, P16, HS], BF16, name="afn_b")
            nc.sync.dma_start(out=afn_b[:], in_=afn_d[:])
            ones_b = cp.tile([1, 128], BF16, name="ones_b")
            nc.vector.memset(ones_b[:], 1.0)
            ident_rep = cp.tile([128, P16, 128], BF16, name="ident_rep")
            nc.vector.tensor_copy(
                ident_rep[:], ident_b[:].unsqueeze(1).broadcast_to([128, P16, 128])
            )
            B_s = cp.tile([128, P16, JS], BF16, name="B_s")

            # ---------------- prologue ----------------
            c_cur = sp.tile([128, HS], F32, name="c", tag="c")
            pay = yp.tile([128, PW], BF16, name="pay", tag="pay")
            with (
                tc.tile_pool(name="prol", bufs=1) as pr,
                tc.tile_pool(name="prolp", bufs=2, space="PSUM") as prp,
            ):
                wat_b = pr.tile([128, KT, JS], BF16, name="wat_b")
                nc.sync.dma_start(out=wat_b[:], in_=wattn_d[:])
                afT_b = pr.tile([128, KT, P16, N], BF16, name="afT_b")
                nc.sync.dma_start(out=afT_b[:], in_=afT_d[:])
                h0_t = pr.tile([128, HS], F32, name="h0_t")
                nc.sync.dma_start(out=h0_t[:], in_=h0_d[:])
                nc.vector.tensor_copy(c_cur[:], h0_t[:])
                h0_b = pr.tile([128, HS], BF16, name="h0_b")
                nc.vector.tensor_copy(h0_b[:], h0_t[:])

                # B[n,p,j] = sum_h Af[n,h,p] Wattn[h,j]
                for p in range(P16):
                    bp = prp.tile([128, JS], F32, name="bp", tag="bp")
                    for kk in range(KT):
                        nc.tensor.matmul(
                            out=bp[:],
                            lhsT=afT_b[:, kk, p, :],
                            rhs=wat_b[:, kk, :],
                            start=(kk == 0), stop=(kk == KT - 1),
                        )
                    nc.scalar.copy(out=B_s[:, p, :], in_=bp[:])

                # payload 0 from h0: [pscores | h0T]
                hTp = tp.tile([128, 128], BF16, name="hTp", tag="tp")
                nc.tensor.transpose(hTp[:], h0_b[:], ident_b[:])
                nc.scalar.copy(out=pay[:, 2 * P16 :], in_=hTp[:])
                prod0 = pr.tile([128, P16, HS], BF16, name="prod0")
                nc.vector.tensor_mul(
                    prod0[:], afn_b[:],
                    h0_b[:].unsqueeze(1).broadcast_to([128, P16, HS]),
                )
                nc.vector.tensor_reduce(
                    pay[:, 0 : 2 * P16].bitcast(F32), prod0[:],
                    mybir.AxisListType.X, mybir.AluOpType.add,
                )

            # ---------------- recurrence ----------------
            def xwx_prefetch(t):
                """x_t @ Wx + b into a fresh act psum bank (during AG window)."""
                xt = xp.tile([128, KT, N], BF16, name="xt", tag="xt")
                nc.sync.dma_start(out=xt[:], in_=xT_d[t])
                act = pp.tile([128, JS], F32, name="act", tag="act")
                for kk in range(KT):
                    nc.tensor.matmul(
                        out=act[:],
                        lhsT=xt[:, kk, :],
                        rhs=wx_b[:, kk, :],
                        start=(kk == 0), stop=False,
                    )
                nc.tensor.matmul(
                    out=act[:], lhsT=ones_b[:], rhs=b_b[:],
                    start=False, stop=False,
                )
                return act

            act_prev = xwx_prefetch(0)

            for t in range(T):
                # ---- ship payload of h_t ----
                bin_ = dp.tile([128, PW], BF16, name="bin", tag="bin")
                nc.sync.dma_start(out=bin_[:], in_=pay[:])
                bout = dp.tile([NC * 128, PW], BF16, addr_space="Shared",
                               name="bout", tag="bout")
                nc.gpsimd.collective_compute(
                    "AllGather", mybir.AluOpType.bypass, replica_groups=RG,
                    ins=[bin_[:].opt()], outs=[bout[:].opt()],
                )
                g_lo = wp.tile([128, 4, PW], BF16, name="g_lo", tag="g_lo")
                g_hi = wp.tile([128, 4, PW], BF16, name="g_hi", tag="g_hi")
                gsrc = bout[:].rearrange("(kk p) f -> p kk f", kk=NC)
                nc.sync.dma_start(out=g_lo[:], in_=gsrc[:, 0:4, :])
                nc.scalar.dma_start(out=g_hi[:], in_=gsrc[:, 4:8, :])

                # ---- softmax weights -> diag halves ----
                s_lo = wp.tile([128, P16], F32, name="s_lo", tag="s_lo")
                s_hi = wp.tile([128, P16], F32, name="s_hi", tag="s_hi")
                nc.vector.tensor_reduce(
                    s_lo[:],
                    g_lo[:, :, 0 : 2 * P16].bitcast(F32).rearrange("n kk q -> n q kk"),
                    mybir.AxisListType.X, mybir.AluOpType.add,
                )
                nc.vector.tensor_reduce(
                    s_hi[:],
                    g_hi[:, :, 0 : 2 * P16].bitcast(F32).rearrange("n kk q -> n q kk"),
                    mybir.AxisListType.X, mybir.AluOpType.add,
                )
                scores = wp.tile([128, P16], F32, name="scores", tag="scores")
                nc.vector.tensor_add(scores[:], s_lo[:], s_hi[:])
                e_b = wp.tile([128, P16], BF16, name="e_b", tag="e_b")
                den = wp.tile([128, 1], F32, name="den", tag="den")
                nc.scalar.activation(
                    out=e_b[:], in_=scores[:], func=mybir.ActivationFunctionType.Exp,
                    scale=1.0 / 32.0, accum_out=den[:],
                )
                rden = wp.tile([128, 1], F32, name="rden", tag="rden")
                nc.vector.reciprocal(rden[:], den[:])
                diag = wp.tile([128, P16, 128], BF16, name="diag", tag="diag")
                for half in range(2):
                    lo, hi = half * 8, half * 8 + 8
                    nc.vector.scalar_tensor_tensor(
                        out=diag[:, lo:hi, :],
                        in0=ident_rep[:, lo:hi, :],
                        scalar=rden[:],
                        in1=e_b[:, lo:hi].unsqueeze(2).broadcast_to([128, 8, 128]),
                        op0=mybir.AluOpType.mult,
                        op1=mybir.AluOpType.mult,
                    )

                # ---- act += h @ Wh + sum_p w_p * B_p ----
                act = act_prev
                for kk in range(KT):
                    gt = g_lo if kk < 4 else g_hi
                    nc.tensor.matmul(
                        out=act[:], lhsT=gt[:, kk % 4, 2 * P16 :], rhs=wh_b[:, kk, :],
                        start=False, stop=False,
                    )
                for p in range(P16):
                    nc.tensor.matmul(
                        out=act[:], lhsT=diag[:, p, :], rhs=B_s[:, p, :],
                        start=False, stop=(p == P16 - 1),
                    )

                # ---- gates (exp/tanh table only) ----
                # th = tanh(act/2); cols 0:384 -> sigmoid halves, 384:512 is
                # tanh(g) directly (g-gate weight columns are pre-doubled).
                th = wp.tile([128, JS], BF16, name="th", tag="th")
                nc.scalar.activation(
                    out=th[:], in_=act[:, 0:JS],
                    func=mybir.ActivationFunctionType.Tanh, scale=0.5,
                )
                sig = wp.tile([128, 3 * HS], BF16, name="sig", tag="sig")
                nc.scalar.activation(
                    out=sig[:], in_=th[:, 0 : 3 * HS],
                    func=mybir.ActivationFunctionType.Copy, bias=0.5, scale=0.5,
                )
                fc = wp.tile([128, HS], F32, name="fc", tag="fc")
                nc.vector.tensor_mul(fc[:], sig[:, HS : 2 * HS], c_cur[:])
                ig = wp.tile([128, HS], F32, name="ig", tag="ig")
                nc.vector.tensor_mul(ig[:], sig[:, 0:HS], th[:, 3 * HS : JS])
                c_new = sp.tile([128, HS], F32, name="c", tag="c")
                nc.vector.tensor_add(c_new[:], fc[:], ig[:])
                tcb = wp.tile([128, HS], BF16, name="tcb", tag="tcb")
                nc.scalar.activation(
                    out=tcb[:], in_=c_new[:], func=mybir.ActivationFunctionType.Tanh,
                )

                # afno = afn (.) o  (runs while scalar does tanh(c))
                afno = wp.tile([128, P16, HS], BF16, name="afno", tag="afno")
                nc.vector.tensor_mul(
                    afno[:], afn_b[:],
                    sig[:, 2 * HS : 3 * HS].unsqueeze(1).broadcast_to([128, P16, HS]),
                )

                # ---- next payload: [pscores(h_new) | h_newT] ----
                # (vector may read only one PSUM operand: stage o^T in SBUF)
                oTp = tp.tile([128, 128], BF16, name="oTp", tag="tp")
                nc.tensor.transpose(oTp[:], sig[:, 2 * HS : 3 * HS], ident_b[:])
                oTs = wp.tile([128, 128], BF16, name="oTs", tag="oTs")
                nc.scalar.copy(out=oTs[:], in_=oTp[:])
                tcTp = tp.tile([128, 128], BF16, name="tcTp", tag="tp")
                nc.tensor.transpose(tcTp[:], tcb[:], ident_b[:])
                pay = yp.tile([128, PW], BF16, name="pay", tag="pay")
                nc.vector.tensor_mul(pay[:, 2 * P16 :], oTs[:], tcTp[:])
                prod = wp.tile([128, P16, HS], BF16, name="prod", tag="prod")
                nc.vector.tensor_mul(
                    prod[:], afno[:],
                    tcb[:].unsqueeze(1).broadcast_to([128, P16, HS]),
                )
                pfold = wp.tile([128, P16, HS // 2], BF16, name="pfold", tag="pfold")
                nc.vector.tensor_add(
                    pfold[:], prod[:, :, 0 : HS // 2], prod[:, :, HS // 2 :]
                )
                nc.vector.tensor_reduce(
                    pay[:, 0 : 2 * P16].bitcast(F32), pfold[:],
                    mybir.AxisListType.X, mybir.AluOpType.add,
                )
                nc.scalar.dma_start(out=out_d[t], in_=pay[:, 2 * P16 :])

                # prefetch next step's x@Wx while the AllGather runs
                if t + 1 < T:
                    act_prev = xwx_prefetch(t + 1)
                c_cur = c_new

    nc.compile()
    return nc


def _get_nc():
    global _nc_cache
    if _nc_cache is None:
        _nc_cache = _build()
    return _nc_cache


def _prep_w(W, k, scale_g):
    """(D|H, 4H) -> [128, KT, JS] bf16, g-gate cols doubled."""
    cols = np.concatenate(
        [W[:, g * H + k * HS : g * H + (k + 1) * HS] * (2.0 if (g == 3 and scale_g) else 1.0)
         for g in range(4)], axis=1)
    return np.ascontiguousarray(
        cols.reshape(KT, 128, JS).transpose(1, 0, 2)).astype(BF16_NP)


def _prepare_in_maps(x, A, Wx, Wh, Wattn, b):
    x = np.asarray(x, dtype=np.float32)
    A = np.asarray(A, dtype=np.float32)
    Wx = np.asarray(Wx, dtype=np.float32)
    Wh = np.asarray(Wh, dtype=np.float32)
    Wattn = np.asarray(Wattn, dtype=np.float32)
    b = np.asarray(b, dtype=np.float32)

    xT = np.ascontiguousarray(
        x.transpose(1, 2, 0).reshape(T, KT, 128, N).transpose(0, 2, 1, 3)
    ).astype(BF16_NP)  # (T, 128, KT, N)
    Af = A.reshape(N, H, P16)
    afT = np.ascontiguousarray(
        Af.transpose(1, 2, 0).reshape(KT, 128, P16, N).transpose(1, 0, 2, 3)
    ).astype(BF16_NP)  # (128, KT, P16, N)
    h0 = A.mean(axis=(2, 3))  # (N, H) f32
    ident = np.eye(128, dtype=np.float32).astype(BF16_NP)

    in_maps = []
    for k in range(NC):
        afn = np.ascontiguousarray(
            Af[:, k * HS : (k + 1) * HS, :].transpose(0, 2, 1)
        ).astype(BF16_NP)  # (N, P16, HS)
        bk = np.concatenate(
            [b[g * H + k * HS : g * H + (k + 1) * HS] * (2.0 if g == 3 else 1.0)
             for g in range(4)])
        in_maps.append({
            "xT": xT,
            "wx": _prep_w(Wx, k, True),
            "wh": _prep_w(Wh, k, True),
            "wattn": _prep_w(Wattn, k, True),
            "bvec": bk.reshape(1, JS).astype(BF16_NP),
            "ident": ident,
            "afn": afn,
            "afT": afT,
            "h0": np.ascontiguousarray(h0[:, k * HS : (k + 1) * HS]),
        })
    return in_maps


def _assemble(results):
    # per-core out: (T, HS, N) -> full (N, T, H)
    full = np.empty((N, T, H), dtype=np.float32)
    for k in range(NC):
        full[:, :, k * HS : (k + 1) * HS] = np.asarray(
            results[k]["out"], dtype=np.float32
        ).transpose(2, 0, 1)
    return full


def kernel(**inputs) -> np.ndarray:
    nc = _get_nc()
    in_maps = _prepare_in_maps(**inputs)
    res = run_bass_kernel_spmd(nc, in_maps, core_ids=list(range(NC)))
    return _assemble(res.results)

